# revision 8
# baseline (speedup 1.0000x reference)
"""BiMamba block Trainium2 kernel: 3 SPMD launches over 8 NeuronCores.

L1 (token-parallel): in_proj, causal depthwise conv (PE diag-matmuls), SiLU,
    x_proj, dt_proj, softplus -> xc/dt/dtu/silu(z)/B/C per (dir,batch) stream.
L2 (dir x batch x channel-half): selective scan via tensor_tensor_scan over
    full L, s-reduction via identity-matmul PSUM accumulation, gating,
    out_proj partial.
L3 (token-parallel): residual combine, LayerNorm, FFN, residual.
"""
import numpy as np
import ml_dtypes
import jax
from jax.sharding import Mesh, PartitionSpec
from jax.experimental.shard_map import shard_map

import concourse.bass as bass
import concourse.tile as tile
import concourse.mybir as mybir
from concourse import bacc
from concourse.bass2jax import _bass_exec_p, install_neuronx_cc_hook, partition_id_tensor

F32 = mybir.dt.float32
BF16 = mybir.dt.bfloat16
BF = ml_dtypes.bfloat16
AF = mybir.ActivationFunctionType
OP = mybir.AluOpType

B, L, DM = 2, 2048, 1024
DI = 2048            # d_inner
DS = 16              # d_state
DTR = 64             # dt_rank
DC = 4               # d_conv
NCORES = 8
HL = L // 2          # tokens per L1 core (1024)
HLP = HL + 3         # with conv halo
HCH = DI // 2        # channels per L2 core (1024)


# ----------------------------------------------------------------- runner ---
class _Runner:
    def __init__(self, nc, n_cores):
        install_neuronx_cc_hook()
        self.n_cores = n_cores
        in_names, out_names, out_avals = [], [], []
        pname = nc.partition_id_tensor.name if nc.partition_id_tensor else None
        for alloc in nc.m.functions[0].allocations:
            if not isinstance(alloc, mybir.MemoryLocationSet):
                continue
            name = alloc.memorylocations[0].name
            if alloc.kind == "ExternalInput":
                if name != pname:
                    in_names.append(name)
            elif alloc.kind == "ExternalOutput":
                out_names.append(name)
                out_avals.append(jax.core.ShapedArray(
                    tuple(alloc.tensor_shape), mybir.dt.np(alloc.dtype)))
        self.in_names, self.out_names, self.out_avals = in_names, out_names, out_avals
        n_params, n_outs = len(in_names), len(out_names)
        all_in = in_names + out_names + ([pname] if pname else [])

        def _body(*args):
            ops = list(args)
            if pname:
                ops.append(partition_id_tensor())
            return tuple(_bass_exec_p.bind(
                *ops, out_avals=tuple(out_avals), in_names=tuple(all_in),
                out_names=tuple(out_names), lowering_input_output_aliases=(),
                sim_require_finite=True, sim_require_nnan=True, nc=nc))

        devices = jax.devices()[:n_cores]
        self.mesh = Mesh(np.asarray(devices), ("core",))
        self.sharding = jax.sharding.NamedSharding(self.mesh, PartitionSpec("core"))
        self._fn = jax.jit(
            shard_map(_body, mesh=self.mesh,
                      in_specs=(PartitionSpec("core"),) * (n_params + n_outs),
                      out_specs=(PartitionSpec("core"),) * n_outs, check_rep=False),
            donate_argnums=tuple(range(n_params, n_params + n_outs)),
            keep_unused=True)

    def run(self, in_maps):
        concat = [jax.device_put(
            np.concatenate([np.asarray(m[n]) for m in in_maps], axis=0),
            self.sharding) for n in self.in_names]
        zeros = [jax.device_put(
            np.zeros((self.n_cores * a.shape[0], *a.shape[1:]), a.dtype),
            self.sharding) for a in self.out_avals]
        outs = self._fn(*concat, *zeros)
        jax.block_until_ready(outs)
        res = [{} for _ in range(self.n_cores)]
        for i, n in enumerate(self.out_names):
            arr = np.asarray(outs[i]).reshape(self.n_cores, *self.out_avals[i].shape)
            for c in range(self.n_cores):
                res[c][n] = arr[c]
        return res


def _mk_nc():
    return bacc.Bacc("TRN2", target_bir_lowering=False, debug=False,
                     num_devices=NCORES)


# --------------------------------------------------------------------- L1 ---
def build_l1():
    nc = _mk_nc()
    xT = nc.dram_tensor("xT", (DM, HLP), BF16, kind="ExternalInput")
    in_wT = nc.dram_tensor("in_wT", (DM, 2 * DI), BF16, kind="ExternalInput")
    x_wT = nc.dram_tensor("x_wT", (DI, DTR + 2 * DS), BF16, kind="ExternalInput")
    dt_wT = nc.dram_tensor("dt_wT", (DTR, DI), BF16, kind="ExternalInput")
    conv_w = nc.dram_tensor("conv_w", (DI, DC), F32, kind="ExternalInput")
    conv_b = nc.dram_tensor("conv_b", (DI, 1), F32, kind="ExternalInput")
    dt_b = nc.dram_tensor("dt_b", (DI, 1), F32, kind="ExternalInput")
    ident = nc.dram_tensor("ident", (128, 128), BF16, kind="ExternalInput")

    o_xc = nc.dram_tensor("o_xc", (DI, HL), BF16, kind="ExternalOutput")
    o_dt = nc.dram_tensor("o_dt", (DI, HL), BF16, kind="ExternalOutput")
    o_dtu = nc.dram_tensor("o_dtu", (DI, HL), BF16, kind="ExternalOutput")
    o_zs = nc.dram_tensor("o_zs", (DI, HL), BF16, kind="ExternalOutput")
    o_bc = nc.dram_tensor("o_bc", (2 * DS, HL), BF16, kind="ExternalOutput")

    NCH = (512, 512, 3)  # column chunks covering HLP=1027

    with tile.TileContext(nc) as tc:
        with (
            tc.tile_pool(name="w", bufs=1) as wp,
            tc.tile_pool(name="work", bufs=3) as work,
            tc.tile_pool(name="big", bufs=1) as big,
            tc.tile_pool(name="psum", bufs=2, space="PSUM") as pp,
        ):
            # weights
            inw = []
            for k in range(8):
                t = wp.tile([128, 2 * DI], BF16, name=f"inw{k}")
                nc.sync.dma_start(t[:], in_wT.ap()[k * 128:(k + 1) * 128, :])
                inw.append(t)
            xt = []
            for k in range(8):
                t = wp.tile([128, HLP], BF16, name=f"xt{k}")
                nc.sync.dma_start(t[:], xT.ap()[k * 128:(k + 1) * 128, :])
                xt.append(t)
            xw = []
            for g in range(16):
                t = wp.tile([128, DTR + 2 * DS], BF16, name=f"xw{g}")
                nc.sync.dma_start(t[:], x_wT.ap()[g * 128:(g + 1) * 128, :])
                xw.append(t)
            dtw = wp.tile([DTR, DI], BF16, name="dtw")
            nc.sync.dma_start(dtw[:], dt_wT.ap())
            cw = wp.tile([128, DC * 16], F32, name="cw")
            for g in range(16):
                nc.sync.dma_start(cw[:, g * DC:(g + 1) * DC],
                                  conv_w.ap()[g * 128:(g + 1) * 128, :])
            cb = wp.tile([128, 16], F32, name="cb")
            dtb = wp.tile([128, 16], F32, name="dtb")
            for g in range(16):
                nc.sync.dma_start(cb[:, g:g + 1], conv_b.ap()[g * 128:(g + 1) * 128, :])
                nc.sync.dma_start(dtb[:, g:g + 1], dt_b.ap()[g * 128:(g + 1) * 128, :])
            idn = wp.tile([128, 128], BF16, name="idn")
            nc.sync.dma_start(idn[:], ident.ap())

            xc = [big.tile([128, HL], BF16, name=f"xc{g}") for g in range(16)]

            # in_proj (xi half) fused with conv, per channel group
            for g in range(16):
                xi = work.tile([128, HLP], BF16, name="xi", bufs=2)
                c0 = 0
                for ncw in NCH:
                    ps = pp.tile([128, 512], F32, name="ps_xz")
                    for kk in range(8):
                        nc.tensor.matmul(
                            ps[:, :ncw], inw[kk][:, g * 128:(g + 1) * 128],
                            xt[kk][:, c0:c0 + ncw], start=(kk == 0), stop=(kk == 7))
                    nc.scalar.copy(xi[:, c0:c0 + ncw], ps[:, :ncw])
                    c0 += ncw
                # conv diag weights for this group
                dia = work.tile([128, DC * 128], BF16, name="dia", bufs=2)
                for k in range(DC):
                    nc.vector.tensor_scalar(
                        dia[:, k * 128:(k + 1) * 128], idn[:],
                        cw[:, g * DC + k:g * DC + k + 1], None, OP.mult)
                for nch in range(2):
                    ps = pp.tile([128, 512], F32, name="ps_conv")
                    for k in range(DC):
                        nc.tensor.matmul(
                            ps[:], dia[:, k * 128:(k + 1) * 128],
                            xi[:, nch * 512 + k:nch * 512 + k + 512],
                            start=(k == 0), stop=(k == DC - 1))
                    nc.scalar.activation(xc[g][:, nch * 512:(nch + 1) * 512],
                                         ps[:], AF.Silu, bias=cb[:, g:g + 1])
                nc.sync.dma_start(o_xc.ap()[g * 128:(g + 1) * 128, :], xc[g][:])

            # in_proj z half: silu -> store
            for mm in range(16, 32):
                c0 = 0
                for ncw in NCH:
                    ps = pp.tile([128, 512], F32, name="ps_xz")
                    for kk in range(8):
                        nc.tensor.matmul(
                            ps[:, :ncw], inw[kk][:, mm * 128:(mm + 1) * 128],
                            xt[kk][:, c0:c0 + ncw], start=(kk == 0), stop=(kk == 7))
                    zt = work.tile([128, 512], BF16, name="zt")
                    nc.scalar.activation(zt[:, :ncw], ps[:, :ncw], AF.Silu)
                    lo, hi = max(c0, 3), c0 + ncw
                    if lo < hi:
                        nc.sync.dma_start(
                            o_zs.ap()[(mm - 16) * 128:(mm - 15) * 128,
                                      lo - 3:hi - 3],
                            zt[:, lo - c0:hi - c0])
                    c0 += ncw

            # x_proj -> xdbl (96, HL)
            xd = big.tile([DTR + 2 * DS, HL], BF16, name="xd")
            for nch in range(2):
                ps = pp.tile([DTR + 2 * DS, 512], F32, name="ps_xd")
                for kk in range(16):
                    nc.tensor.matmul(ps[:], xw[kk][:],
                                     xc[kk][:, nch * 512:(nch + 1) * 512],
                                     start=(kk == 0), stop=(kk == 15))
                nc.scalar.copy(xd[:, nch * 512:(nch + 1) * 512], ps[:])
            nc.sync.dma_start(o_bc.ap(), xd[DTR:DTR + 2 * DS, :])

            # dt_proj + softplus -> dt ; dtu = dt * xc
            for g in range(16):
                dtg = work.tile([128, HL], BF16, name="dtg")
                for nch in range(2):
                    ps = pp.tile([128, 512], F32, name="ps_dt")
                    nc.tensor.matmul(ps[:], dtw[:, g * 128:(g + 1) * 128],
                                     xd[0:DTR, nch * 512:(nch + 1) * 512],
                                     start=True, stop=True)
                    # softplus(x) = ln(exp(x) + 1)
                    ex = work.tile([128, 512], F32, name="ex")
                    nc.scalar.activation(ex[:], ps[:], AF.Exp,
                                         bias=dtb[:, g:g + 1])
                    nc.scalar.activation(dtg[:, nch * 512:(nch + 1) * 512],
                                         ex[:], AF.Ln, bias=1.0)
                nc.sync.dma_start(o_dt.ap()[g * 128:(g + 1) * 128, :], dtg[:])
                dtug = work.tile([128, HL], BF16, name="dtug")
                nc.vector.tensor_tensor(dtug[:], dtg[:], xc[g][:], op=OP.mult)
                nc.sync.dma_start(o_dtu.ap()[g * 128:(g + 1) * 128, :], dtug[:])
    nc.compile()
    return nc


# --------------------------------------------------------------------- L2 ---
def build_l2():
    nc = _mk_nc()
    ins = {}
    for nm in ("dt", "dtu", "xc", "zs"):
        for h in (0, 1):
            ins[f"{nm}{h}"] = nc.dram_tensor(f"{nm}{h}", (HCH, HL), BF16,
                                             kind="ExternalInput")
    Bf = nc.dram_tensor("Bf", (DS, L), BF16, kind="ExternalInput")
    Cf = nc.dram_tensor("Cf", (DS, L), BF16, kind="ExternalInput")
    Ar = nc.dram_tensor("Ar", (128, 128), F32, kind="ExternalInput")
    Dr = nc.dram_tensor("Dr", (128, 8), F32, kind="ExternalInput")
    owT = nc.dram_tensor("owT", (HCH, DM), BF16, kind="ExternalInput")
    ident = nc.dram_tensor("ident", (128, 128), BF16, kind="ExternalInput")
    part = nc.dram_tensor("part", (DM, L), F32, kind="ExternalOutput")

    def gsl(g):
        return slice(g * 128, (g + 1) * 128)

    with tile.TileContext(nc) as tc:
        with (
            tc.tile_pool(name="w", bufs=1) as wp,
            tc.tile_pool(name="bc", bufs=3) as bcp,
            tc.tile_pool(name="work", bufs=3) as work,
            tc.tile_pool(name="y", bufs=1) as yp,
            tc.tile_pool(name="psy", bufs=1, space="PSUM") as psy,
            tc.tile_pool(name="pso", bufs=4, space="PSUM") as pso,
        ):
            Art = wp.tile([128, 128], F32, name="Art")
            nc.sync.dma_start(Art[:], Ar.ap())
            Drt = wp.tile([128, 8], F32, name="Drt")
            nc.sync.dma_start(Drt[:], Dr.ap())
            idn = wp.tile([128, 128], BF16, name="idn")
            nc.sync.dma_start(idn[:], ident.ap())
            ow = []
            for k in range(8):
                t = wp.tile([128, DM], BF16, name=f"ow{k}")
                nc.sync.dma_start(t[:], owT.ap()[k * 128:(k + 1) * 128, :])
                ow.append(t)

            ys = []
            for g in range(8):
                dtg = work.tile([128, L], BF16, name="dtg", bufs=2)
                dtug = work.tile([128, L], BF16, name="dtug", bufs=2)
                for h in (0, 1):
                    nc.sync.dma_start(dtg[:, h * HL:(h + 1) * HL],
                                      ins[f"dt{h}"].ap()[gsl(g), :])
                    nc.sync.dma_start(dtug[:, h * HL:(h + 1) * HL],
                                      ins[f"dtu{h}"].ap()[gsl(g), :])
                psum_y = psy.tile([128, L], F32, name="psum_y")
                for s in range(DS):
                    bbc = bcp.tile([128, L], BF16, name="bbc")
                    nc.sync.dma_start(bbc[:], Bf.ap()[s:s + 1, :].broadcast_to((128, L)))
                    cbc = bcp.tile([128, L], BF16, name="cbc")
                    nc.sync.dma_start(cbc[:], Cf.ap()[s:s + 1, :].broadcast_to((128, L)))
                    a_t = work.tile([128, L], BF16, name="a_t")
                    nc.scalar.activation(a_t[:], dtg[:], AF.Exp,
                                         scale=Art[:, g * 16 + s:g * 16 + s + 1])
                    b_t = work.tile([128, L], BF16, name="b_t")
                    nc.vector.tensor_tensor(b_t[:], dtug[:], bbc[:], op=OP.mult)
                    h_t = work.tile([128, L], BF16, name="h_t")
                    nc.vector.tensor_tensor_scan(h_t[:], a_t[:], b_t[:], 0.0,
                                                 op0=OP.mult, op1=OP.add)
                    hc_t = work.tile([128, L], BF16, name="hc_t")
                    nc.vector.tensor_tensor(hc_t[:], h_t[:], cbc[:], op=OP.mult)
                    for j in range(4):
                        nc.tensor.matmul(psum_y[:, j * 512:(j + 1) * 512], idn[:],
                                         hc_t[:, j * 512:(j + 1) * 512],
                                         start=(s == 0), stop=False)
                # D-term: psum_y += diag(D_g) @ xc_g
                xcg = work.tile([128, L], BF16, name="xcg", bufs=2)
                zsg = work.tile([128, L], BF16, name="zsg", bufs=2)
                for h in (0, 1):
                    nc.sync.dma_start(xcg[:, h * HL:(h + 1) * HL],
                                      ins[f"xc{h}"].ap()[gsl(g), :])
                    nc.sync.dma_start(zsg[:, h * HL:(h + 1) * HL],
                                      ins[f"zs{h}"].ap()[gsl(g), :])
                dg = work.tile([128, 128], BF16, name="dg", bufs=2)
                nc.vector.tensor_scalar(dg[:], idn[:], Drt[:, g:g + 1], None,
                                        OP.mult)
                for j in range(4):
                    nc.tensor.matmul(psum_y[:, j * 512:(j + 1) * 512], dg[:],
                                     xcg[:, j * 512:(j + 1) * 512],
                                     start=False, stop=True)
                yg = yp.tile([128, L], BF16, name=f"yg{g}")
                nc.vector.tensor_tensor(yg[:], psum_y[:], zsg[:], op=OP.mult)
                ys.append(yg)

            # out_proj partial: part[dm, t] = sum_ch owT[ch, dm] * y[ch, t]
            for mm in range(8):
                for nch in range(4):
                    ps = pso.tile([128, 512], F32, name="ps_o")
                    for kk in range(8):
                        nc.tensor.matmul(ps[:], ow[kk][:, mm * 128:(mm + 1) * 128],
                                         ys[kk][:, nch * 512:(nch + 1) * 512],
                                         start=(kk == 0), stop=(kk == 7))
                    ot = work.tile([128, 512], F32, name="ot")
                    nc.scalar.copy(ot[:], ps[:])
                    nc.sync.dma_start(
                        part.ap()[mm * 128:(mm + 1) * 128,
                                  nch * 512:(nch + 1) * 512], ot[:])
    nc.compile()
    return nc


# --------------------------------------------------------------------- L3 ---
def build_l3():
    nc = _mk_nc()
    TQ = L // 4  # 512 tokens per core
    xTs = nc.dram_tensor("xTs", (DM, TQ), F32, kind="ExternalInput")
    parts = [nc.dram_tensor(f"p{i}", (DM, TQ), F32, kind="ExternalInput")
             for i in range(4)]
    w1T = nc.dram_tensor("w1T", (DM, 4 * DM), BF16, kind="ExternalInput")
    w2T = nc.dram_tensor("w2T", (4 * DM, DM), BF16, kind="ExternalInput")
    b1 = nc.dram_tensor("b1", (4 * DM, 1), F32, kind="ExternalInput")
    b2 = nc.dram_tensor("b2", (DM, 1), F32, kind="ExternalInput")
    lng = nc.dram_tensor("lng", (DM, 1), F32, kind="ExternalInput")
    lnb = nc.dram_tensor("lnb", (DM, 1), F32, kind="ExternalInput")
    out = nc.dram_tensor("out", (TQ, DM), F32, kind="ExternalOutput")

    with tile.TileContext(nc) as tc:
        with (
            tc.tile_pool(name="w", bufs=1) as wp,
            tc.tile_pool(name="work", bufs=2) as work,
            tc.tile_pool(name="h1p", bufs=1) as h1p,
        ):
            w1 = []
            for k in range(8):
                t = wp.tile([128, 4 * DM], BF16, name=f"w1_{k}")
                nc.sync.dma_start(t[:], w1T.ap()[k * 128:(k + 1) * 128, :])
                w1.append(t)
            b1t = wp.tile([128, 32], F32, name="b1t")
            for g in range(32):
                nc.sync.dma_start(b1t[:, g:g + 1], b1.ap()[g * 128:(g + 1) * 128, :])
            b2t = wp.tile([128, 8], F32, name="b2t")
            lgt = wp.tile([128, 8], F32, name="lgt")
            lbt = wp.tile([128, 8], F32, name="lbt")
            for g in range(8):
                nc.sync.dma_start(b2t[:, g:g + 1], b2.ap()[g * 128:(g + 1) * 128, :])
                nc.sync.dma_start(lgt[:, g:g + 1], lng.ap()[g * 128:(g + 1) * 128, :])
                nc.sync.dma_start(lbt[:, g:g + 1], lnb.ap()[g * 128:(g + 1) * 128, :])
            ones1 = wp.tile([128, 1], BF16, name="ones1")
            nc.vector.memset(ones1[:], 1.0)
            onesr = wp.tile([1, 128], BF16, name="onesr")
            nc.vector.memset(onesr[:], 1.0)

            # x_new = x + 0.5*(p0+p1+p2+p3), per 128-row group
            xn = []
            for g in range(8):
                pt = [work.tile([128, TQ], F32, name=f"pt{i}", bufs=2) for i in range(4)]
                for i in range(4):
                    nc.sync.dma_start(pt[i][:], parts[i].ap()[g * 128:(g + 1) * 128, :])
                xt = work.tile([128, TQ], F32, name="xt")
                nc.sync.dma_start(xt[:], xTs.ap()[g * 128:(g + 1) * 128, :])
                s1 = work.tile([128, TQ], F32, name="s1")
                nc.vector.tensor_tensor(s1[:], pt[0][:], pt[1][:], op=OP.add)
                s2 = work.tile([128, TQ], F32, name="s2")
                nc.vector.tensor_tensor(s2[:], pt[2][:], pt[3][:], op=OP.add)
                nc.vector.tensor_tensor(s1[:], s1[:], s2[:], op=OP.add)
                xg = wp.tile([128, TQ], F32, name=f"xn{g}")
                nc.vector.scalar_tensor_tensor(xg[:], s1[:], 0.5, xt[:],
                                               op0=OP.mult, op1=OP.add)
                xn.append(xg)

            # LayerNorm over dm (partition dim) via ones-matmul reductions
            hn = []
            with tc.tile_pool(name="pst", bufs=1, space="PSUM") as pst:
                ps_s = pst.tile([1, TQ], F32, name="ps_s")
                xnb = []
                for g in range(8):
                    t = work.tile([128, TQ], BF16, name="xnb", bufs=4)
                    nc.vector.tensor_copy(t[:], xn[g][:])
                    xnb.append(t)
                for g in range(8):
                    nc.tensor.matmul(ps_s[:], ones1[:], xnb[g][:],
                                     start=(g == 0), stop=(g == 7))
                ps_q = pst.tile([1, TQ], F32, name="ps_q")
                sq = []
                for g in range(8):
                    t = work.tile([128, TQ], BF16, name="sq", bufs=4)
                    nc.scalar.activation(t[:], xn[g][:], AF.Square)
                    sq.append(t)
                for g in range(8):
                    nc.tensor.matmul(ps_q[:], ones1[:], sq[g][:],
                                     start=(g == 0), stop=(g == 7))
                mean = wp.tile([1, TQ], F32, name="mean")
                nc.scalar.activation(mean[:], ps_s[:], AF.Copy, scale=1.0 / DM)
                msq = work.tile([1, TQ], F32, name="msq")
                nc.scalar.activation(msq[:], mean[:], AF.Square)
                var = wp.tile([1, TQ], F32, name="var")
                nc.vector.scalar_tensor_tensor(var[:], ps_q[:], 1.0 / DM, msq[:],
                                               op0=OP.mult, op1=OP.subtract)
                eps = wp.tile([1, 1], F32, name="eps")
                nc.vector.memset(eps[:], 1e-5)
                std = work.tile([1, TQ], F32, name="std")
                nc.scalar.activation(std[:], var[:], AF.Sqrt, bias=eps[:])
                rstd = wp.tile([1, TQ], F32, name="rstd")
                nc.vector.reciprocal(rstd[:], std[:])
                rstdb = wp.tile([1, TQ], BF16, name="rstdb")
                nc.vector.tensor_copy(rstdb[:], rstd[:])
                meanb = wp.tile([1, TQ], BF16, name="meanb")
                nc.vector.tensor_copy(meanb[:], mean[:])
                # broadcast to 128 partitions via k=1 ones matmul
                ps_mb = pst.tile([128, TQ], F32, name="ps_mb")
                nc.tensor.matmul(ps_mb[:], onesr[:], meanb[:], start=True, stop=True)
                ps_rb = pst.tile([128, TQ], F32, name="ps_rb")
                nc.tensor.matmul(ps_rb[:], onesr[:], rstdb[:], start=True, stop=True)
                mb = wp.tile([128, TQ], F32, name="mb")
                nc.scalar.copy(mb[:], ps_mb[:])
                rb = wp.tile([128, TQ], F32, name="rb")
                nc.scalar.copy(rb[:], ps_rb[:])

                for g in range(8):
                    t1 = work.tile([128, TQ], F32, name="t1")
                    nc.vector.tensor_tensor(t1[:], xn[g][:], mb[:], op=OP.subtract)
                    nc.vector.tensor_tensor(t1[:], t1[:], rb[:], op=OP.mult)
                    hg = wp.tile([128, TQ], BF16, name=f"hn{g}")
                    nc.vector.tensor_scalar(hg[:], t1[:], lgt[:, g:g + 1],
                                            lbt[:, g:g + 1], OP.mult, OP.add)
                    hn.append(hg)

            # FFN mm1 + silu
            h1 = [h1p.tile([128, TQ], BF16, name=f"h1_{m}") for m in range(32)]
            with tc.tile_pool(name="ps1", bufs=4, space="PSUM") as ps1p:
                for mm in range(32):
                    ps = ps1p.tile([128, 512], F32, name="ps_1")
                    for kk in range(8):
                        nc.tensor.matmul(ps[:], w1[kk][:, mm * 128:(mm + 1) * 128],
                                         hn[kk][:], start=(kk == 0), stop=(kk == 7))
                    nc.scalar.activation(h1[mm][:], ps[:], AF.Silu,
                                         bias=b1t[:, mm:mm + 1])
            # FFN mm2 (kk-outer, streamed w2) + bias + residual
            with tc.tile_pool(name="ps2", bufs=1, space="PSUM") as ps2p:
                ps2 = [ps2p.tile([128, TQ], F32, name=f"ps_2_{m}") for m in range(8)]
                for kk in range(32):
                    w2t = work.tile([128, DM], BF16, name="w2t", bufs=2)
                    nc.sync.dma_start(w2t[:], w2T.ap()[kk * 128:(kk + 1) * 128, :])
                    for mm in range(8):
                        nc.tensor.matmul(ps2[mm][:],
                                         w2t[:, mm * 128:(mm + 1) * 128],
                                         h1[kk][:], start=(kk == 0),
                                         stop=(kk == 31))
                for mm in range(8):
                    ot = work.tile([128, TQ], F32, name="ot")
                    nc.vector.tensor_scalar(ot[:], ps2[mm][:], b2t[:, mm:mm + 1],
                                            None, OP.add)
                    nc.vector.tensor_tensor(ot[:], ot[:], xn[mm][:], op=OP.add)
                    # store transposed: out[t, dm]
                    nc.sync.dma_start(
                        out.ap()[:, mm * 128:(mm + 1) * 128].transpose([1, 0]), ot[:])
    nc.compile()
    return nc


# ------------------------------------------------------------------- host ---
_CACHE = {}


def _get(name):
    if name not in _CACHE:
        nc = {"l1": build_l1, "l2": build_l2, "l3": build_l3}[name]()
        _CACHE[name] = _Runner(nc, NCORES)
    return _CACHE[name]


def _prep_dir(inp, d):
    f32 = np.float32
    return {
        "in_wT": np.ascontiguousarray(np.asarray(inp[f"{d}_in_w"], f32).T).astype(BF),
        "x_wT": np.ascontiguousarray(np.asarray(inp[f"{d}_x_w"], f32).T).astype(BF),
        "dt_wT": np.ascontiguousarray(np.asarray(inp[f"{d}_dt_w"], f32).T).astype(BF),
        "conv_w": np.asarray(inp[f"{d}_conv_w"], f32),
        "conv_b": np.asarray(inp[f"{d}_conv_b"], f32).reshape(DI, 1),
        "dt_b": np.asarray(inp[f"{d}_dt_b"], f32).reshape(DI, 1),
        "A": -np.exp(np.asarray(inp[f"{d}_A_log"], np.float64)).astype(f32),
        "D": np.asarray(inp[f"{d}_D"], f32),
        "owT": np.ascontiguousarray(np.asarray(inp[f"{d}_out_w"], f32).T).astype(BF),
    }


STREAMS = [("fwd", 0), ("fwd", 1), ("bwd", 0), ("bwd", 1)]


def kernel(**inp):
    f32 = np.float32
    x = np.asarray(inp["x"], f32)
    ident = np.eye(128, dtype=BF)
    prep = {d: _prep_dir(inp, d) for d in ("fwd", "bwd")}

    xT = {}
    for d, b in STREAMS:
        xt = x[b].T if d == "fwd" else x[b, ::-1].T
        pad = np.zeros((DM, L + 3), f32)
        pad[:, 3:] = xt
        xT[(d, b)] = pad.astype(BF)

    # ---- L1 ----
    l1_maps = []
    for d, b in STREAMS:
        for hf in (0, 1):
            p = prep[d]
            l1_maps.append({
                "xT": np.ascontiguousarray(xT[(d, b)][:, hf * HL:hf * HL + HLP]),
                "in_wT": p["in_wT"], "x_wT": p["x_wT"], "dt_wT": p["dt_wT"],
                "conv_w": p["conv_w"], "conv_b": p["conv_b"], "dt_b": p["dt_b"],
                "ident": ident,
            })
    r1 = _get("l1").run(l1_maps)

    # ---- L2 ----
    l2_maps = []
    for si, (d, b) in enumerate(STREAMS):
        p = prep[d]
        h0, h1 = r1[si * 2], r1[si * 2 + 1]
        Bfull = np.concatenate([h0["o_bc"][0:DS], h1["o_bc"][0:DS]], axis=1)
        Cfull = np.concatenate([h0["o_bc"][DS:], h1["o_bc"][DS:]], axis=1)
        for half in (0, 1):
            c0 = half * HCH
            Ar = p["A"][c0:c0 + HCH].reshape(8, 128, DS).transpose(1, 0, 2).reshape(128, 128)
            Dr = p["D"][c0:c0 + HCH].reshape(8, 128).T
            l2_maps.append({
                "dt0": h0["o_dt"][c0:c0 + HCH], "dt1": h1["o_dt"][c0:c0 + HCH],
                "dtu0": h0["o_dtu"][c0:c0 + HCH], "dtu1": h1["o_dtu"][c0:c0 + HCH],
                "xc0": h0["o_xc"][c0:c0 + HCH], "xc1": h1["o_xc"][c0:c0 + HCH],
                "zs0": h0["o_zs"][c0:c0 + HCH], "zs1": h1["o_zs"][c0:c0 + HCH],
                "Bf": Bfull, "Cf": Cfull,
                "Ar": np.ascontiguousarray(Ar), "Dr": np.ascontiguousarray(Dr),
                "owT": p["owT"][c0:c0 + HCH], "ident": ident,
            })
    r2 = _get("l2").run(l2_maps)

    # ---- L3 ----
    TQ = L // 4
    pb = {}
    for si, (d, b) in enumerate(STREAMS):
        for half in (0, 1):
            part = r2[si * 2 + half]["part"]
            if d == "bwd":
                part = np.ascontiguousarray(part[:, ::-1])
            pb[(d, b, half)] = part
    xTf = {b: np.ascontiguousarray(x[b].T) for b in range(B)}
    w1T = np.ascontiguousarray(np.asarray(inp["ff_w1"], f32).T).astype(BF)
    w2T = np.ascontiguousarray(np.asarray(inp["ff_w2"], f32).T).astype(BF)
    l3_maps = []
    for b in range(B):
        for q in range(4):
            sl = slice(q * TQ, (q + 1) * TQ)
            l3_maps.append({
                "xTs": np.ascontiguousarray(xTf[b][:, sl]),
                "p0": np.ascontiguousarray(pb[("fwd", b, 0)][:, sl]),
                "p1": np.ascontiguousarray(pb[("fwd", b, 1)][:, sl]),
                "p2": np.ascontiguousarray(pb[("bwd", b, 0)][:, sl]),
                "p3": np.ascontiguousarray(pb[("bwd", b, 1)][:, sl]),
                "w1T": w1T, "w2T": w2T,
                "b1": np.asarray(inp["ff_b1"], f32).reshape(-1, 1),
                "b2": np.asarray(inp["ff_b2"], f32).reshape(-1, 1),
                "lng": np.asarray(inp["ff_ln_g"], f32).reshape(-1, 1),
                "lnb": np.asarray(inp["ff_ln_b"], f32).reshape(-1, 1),
            })
    r3 = _get("l3").run(l3_maps)
    out = np.concatenate([r3[c]["out"] for c in range(NCORES)], axis=0)
    return out.reshape(B, L, DM).astype(np.float32)
